# revision 42
# baseline (speedup 1.0000x reference)
"""Trainium2 Bass kernel for a dense cross-task transformer block (v2: fp8).

Math notes
----------
The reference "attention" has sequence length 1 on the key axis, so
softmax(scores) == 1.0 exactly and the whole q/k/score path is dead:

    mha_len1(q_in, kv_in, ...) == (kv_in @ wv.T + bv) @ wo.T + bo

which folds (on host) into a single matmul with W = wo @ wv and
b = wo @ bv + bo.  The block is then:

    verb1 = LN(verb + noun @ W1.T + c1)          (ln_v)
    verb2 = verb1 + FFN_v(verb1)
    noun1 = LN(noun + verb2 @ W2.T + c2)         (ln_n)
    noun2 = noun1 + FFN_n(noun1)
    return verb2, noun2

Bias folding: nT is pre-loaded as 16*(noun + c2) so it serves both as the
phase-C residual (which needs +c2) and the phase-A rhs; the resulting
spurious W1@c2 term in phase A is cancelled by pre-subtracting W1@c2 from
the verb residual on the host.  (All biases are zero in practice.)

Precision scheme (the kernel is PE-issue-bound, so FLOP dtype = speed):
 - Everything stored/streamed in fp16 (not bf16: 8x less rounding noise),
   activations and residuals carried at 16x scale so fp8-weight matmuls,
   f16-weight matmuls and residual adds all share one PSUM scale.  LN is
   scale-invariant, outputs are divided by 16 on the host.
 - FFN1 runs fully in fp8 e4m3 DoubleRow (2 contraction rows/cycle):
   lhsT = fp8(16*w1) pairs, rhs = fp8(LN output, true scale).
 - FFN2 contracts 2048: first F2P*256 rows in fp8 DoubleRow (gelu evacuates
   those h-tiles straight to fp8 pair tiles), the rest in f16.
 - Attention GEMMs stay f16 (their quantization noise is amplified ~2x
   through the LN + downstream path; FFN noise is the cheapest to spend).
   The fp8/f16 mix is chosen so total rel-err ~1.7e-2 < 2e-2 budget.

Engine choreography:
 - attention epilogues (PSUM evac, x^2, LN stats matmuls) are emitted one
   m-tile late so the PE never waits on DVE/ACT; stats PSUM is
   double-buffered across chunks (sx at partition 0, sqs at partition 32
   of one bank) so chunk c+1 never WARs on chunk c's stats reads.
 - LN mean/rstd broadcast via rank-1 matmuls, copied to f16 SBUF; the
   per-tile LN tail is 3 DVE ops + 1 ACT fp8 cast.
 - FFN weights stream per m-block; FFN1/FFN2 process chunk-pairs so each
   DoubleRow LDWEIGHTS (213ns, no FWL) amortizes over 2 matmuls.
"""

import os
import numpy as np
import ml_dtypes
from contextlib import ExitStack

import concourse.bass as bass
import concourse.bacc as bacc_mod
import concourse.mybir as mybir
import concourse.tile as tile
from concourse.bass_utils import run_bass_kernel_spmd

E = 1024          # embed dim
H2 = 2048         # FFN hidden dim
B_TOTAL = 16384
NCORES = 8
B = B_TOTAL // NCORES   # 2048 rows per core
P = 128
EPS = 1e-5
CH = 512          # column chunk
NCH = B // CH     # 4
KT = E // P       # 8
MT = E // P       # 8
HT = H2 // P      # 16
S = 16.0          # global activation/residual scale
F1P = 4           # FFN1 fp8 pairs (of KT//2=4): fully fp8
F2PV = 6          # verb FFN2 fp8 pairs (of HT//2=8)
F2PN = 5          # noun FFN2 fp8 pairs
F2PMAX = max(F2PV, F2PN)

F32 = mybir.dt.float32
F16 = mybir.dt.float16
F8 = mybir.dt.float8e4
AF = mybir.ActivationFunctionType
OP = mybir.AluOpType
DR = mybir.MatmulPerfMode.DoubleRow


def _build_program():
    nc = bacc_mod.Bacc("TRN2", target_bir_lowering=False)

    # activations/attention weights laid out [P, KT, cols] so bulk loads can
    # be single multi-k-tile DMA descriptors (startup is descriptor-bound)
    vT = nc.declare_dram_parameter("vT", [P, KT, B], F16, isOutput=False)
    nT = nc.declare_dram_parameter("nT", [P, KT, B], F16, isOutput=False)
    wvo1 = nc.declare_dram_parameter("wvo1", [P, KT, E], F16, isOutput=False)
    wvo2 = nc.declare_dram_parameter("wvo2", [P, KT, E], F16, isOutput=False)
    lnv = nc.declare_dram_parameter("lnv", [P, 2 * MT], F32, isOutput=False)  # 16g|16b
    lnn = nc.declare_dram_parameter("lnn", [P, 2 * MT], F32, isOutput=False)
    w1qv = nc.declare_dram_parameter("w1qv", [F1P * P, 2, H2], F8, isOutput=False)
    w1qn = nc.declare_dram_parameter("w1qn", [F1P * P, 2, H2], F8, isOutput=False)
    b1v = nc.declare_dram_parameter("b1v", [P, HT], F32, isOutput=False)
    b1n = nc.declare_dram_parameter("b1n", [P, HT], F32, isOutput=False)
    w2qv = nc.declare_dram_parameter("w2qv", [MT * P, 2 * F2PV, P], F8, isOutput=False)
    w2qn = nc.declare_dram_parameter("w2qn", [MT * P, 2 * F2PN, P], F8, isOutput=False)
    w2hv = nc.declare_dram_parameter("w2hv", [MT * P, HT - 2 * F2PV, P], F16,
                                     isOutput=False)
    w2hn = nc.declare_dram_parameter("w2hn", [MT * P, HT - 2 * F2PN, P], F16,
                                     isOutput=False)
    b2v = nc.declare_dram_parameter("b2v", [P, MT], F32, isOutput=False)  # 16*b2
    b2n = nc.declare_dram_parameter("b2n", [P, MT], F32, isOutput=False)
    verb_out = nc.declare_dram_parameter("verb_out", [E, B], F16, isOutput=True)
    noun_out = nc.declare_dram_parameter("noun_out", [E, B], F16, isOutput=True)

    with tile.TileContext(nc) as tc, ExitStack() as ctx:
        const = ctx.enter_context(tc.tile_pool(name="const", bufs=1))
        rhsp = ctx.enter_context(tc.tile_pool(name="rhsp", bufs=1))    # nT; later noun2'
        resp = ctx.enter_context(tc.tile_pool(name="resp", bufs=1))    # vT; later verb2'
        lnp = ctx.enter_context(tc.tile_pool(name="lnp", bufs=1))      # verb1'/noun1' f16
        l8p = ctx.enter_context(tc.tile_pool(name="l8p", bufs=1))      # verb1/noun1 fp8 pairs
        watp = ctx.enter_context(tc.tile_pool(name="watp", bufs=1))    # wvo1/wvo2
        w1p = ctx.enter_context(tc.tile_pool(name="w1p", bufs=1))      # w1 fp8 pair tiles
        w2qp = ctx.enter_context(tc.tile_pool(name="w2qp", bufs=3))    # w2 fp8 m-block stream
        w2hp = ctx.enter_context(tc.tile_pool(name="w2hp", bufs=3))    # w2 f16 m-block stream
        hp = ctx.enter_context(tc.tile_pool(name="hp", bufs=1))        # gelu hidden
        sqp = ctx.enter_context(tc.tile_pool(name="sqp", bufs=1))      # x^2 (8 tags)
        smp = ctx.enter_context(tc.tile_pool(name="smp", bufs=1))      # LN stats vectors
        bsp = ctx.enter_context(tc.tile_pool(name="bsp", bufs=3))      # bcast SBUF copies
        mmp = ctx.enter_context(tc.tile_pool(name="mmp", bufs=4, space="PSUM"))
        stp = ctx.enter_context(tc.tile_pool(name="stp", bufs=2, space="PSUM"))
        bcp = ctx.enter_context(tc.tile_pool(name="bcp", bufs=1, space="PSUM"))

        ones_col = const.tile([P, 1], F16, tag="ones_col", name="ones_col")
        nc.vector.memset(ones_col[:], 1.0)
        ones_row = const.tile([1, P], F16, tag="ones_row", name="ones_row")
        nc.vector.memset(ones_row[:], 1.0)
        eps_t = const.tile([1, 1], F32, tag="eps", name="eps")
        nc.vector.memset(eps_t[:], EPS * S * S)

        def load_const(dram, shape, tag, dtype=F32):
            t = const.tile(shape, dtype, tag=tag, name=tag)
            nc.sync.dma_start(out=t[:], in_=dram[:, :])
            return t

        def chunk_tiles(pool, pfx):
            return [[pool.tile([P, CH], F16, tag=f"{pfx}{k}c{c}", name=f"{pfx}{k}c{c}")
                     for c in range(NCH)] for k in range(KT)]

        def pair_tiles(pool, pfx):
            return [pool.tile([P, 2, B], F8, tag=f"{pfx}{kk}", name=f"{pfx}{kk}")
                    for kk in range(KT // 2)]

        def full_tiles(pool, pfx):
            # one big tile so bulk loads can be single multi-k DMA descriptors
            big = pool.tile([P, KT, B], F16, tag=f"{pfx}big", name=f"{pfx}big")
            return [big[:, k, :] for k in range(KT)], big

        def load_wat(dram):
            big = watp.tile([P, KT, E], F16, tag="wabig", name="wabig")
            half = KT // 2
            nc.scalar.dma_start(out=big[:, 0:half, :], in_=dram[:, 0:half, :])
            nc.sync.dma_start(out=big[:, half:KT, :], in_=dram[:, half:KT, :])
            return [big[:, k, :] for k in range(KT)]

        def load_w1q(dram):
            tiles = []
            for kk in range(F1P):
                t = w1p.tile([P, 2, H2], F8, tag=f"w1_{kk}", name=f"w1_{kk}")
                eng = nc.scalar if kk % 2 == 0 else nc.sync
                eng.dma_start(out=t[:, :, :], in_=dram[kk * P:(kk + 1) * P, :, :])
                tiles.append(t)
            return tiles

        def attn_phase(wt, rhs, resid, out16t, out8t, gb_pb, k_outer_c0=False):
            """resid[m][:,cs] <- xt' = resid + wt.T@rhs (16x, f16, in place);
            out16t[m][c] <- 16*LN(x) f16; out8t[kk][:,m%2,cs] <- fp8 LN(x).

            Pipelined epilogues: the PSUM-evac add trails its matmul group by
            one m-slot, the x^2 + stats matmuls by two (so the PE's in-order
            queue never waits on DVE/ACT), and the per-chunk LN post work is
            sliced per-m and drip-fed one item per slot.  Unfinished items are
            returned as a thunk list the next phase interleaves into itself.
            """
            g_pb = gb_pb[:, 0:MT]
            b_pb = gb_pb[:, MT:2 * MT]
            seq = [(c, m) for c in range(NCH) for m in range(MT)]
            stash = {}
            headq = []
            tailq = []

            def add_item(s):
                c, m, ps, st = stash[s]
                cs = slice(c * CH, (c + 1) * CH)
                xt = resid[m][:, cs]
                nc.vector.tensor_add(xt, ps[:], xt)

            def sqstat_item(s):
                # x^2 per m-slot (spread over the ACT/DVE queues); the 16
                # stats matmuls are emitted as one per-chunk batch so the PE
                # pays the array-reconfig hiccup twice per chunk, not 16x.
                c, m, ps, st = stash.pop(s)
                cs = slice(c * CH, (c + 1) * CH)
                xt = resid[m][:, cs]
                sq = sqp.tile([P, CH], F16, tag=f"sq{m}", name=f"sq{m}")
                sqt[m] = sq
                if flush_mode[0]:
                    nc.vector.tensor_mul(sq[:], xt, xt)
                else:
                    nc.scalar.activation(sq[:], xt, AF.Square)
                if m == MT - 1:
                    headq.insert(0, lambda: stats_batch(c, st))

            def stats_batch(c, st):
                cs = slice(c * CH, (c + 1) * CH)
                for m in range(MT):
                    nc.tensor.matmul(st[0:1, :], lhsT=ones_col[:],
                                     rhs=resid[m][:, cs],
                                     start=(m == 0), stop=(m == MT - 1))
                    nc.tensor.matmul(st[32:33, :], lhsT=ones_col[:],
                                     rhs=sqt[m][:],
                                     start=(m == 0), stop=(m == MT - 1))
                headq.append(lambda: post_head(c, st))
                for mm in range(MT):
                    tailq.append(lambda c=c, mm=mm: post_tail(c, mm))

            sqt = {}

            def post_bcast(c, nm, rsb):
                dve = flush_mode[0]
                nmB = bcp.tile([P, CH], F32, tag="nmB", name="nmB")
                nc.tensor.matmul(nmB[:], lhsT=ones_row[:], rhs=nm[:],
                                 start=True, stop=True)
                rsB = bcp.tile([P, CH], F32, tag="rsB", name="rsB")
                nc.tensor.matmul(rsB[:], lhsT=ones_row[:], rhs=rsb[:],
                                 start=True, stop=True)
                nmS = bsp.tile([P, CH], F16, tag="nmS", name="nmS")
                rsS = bsp.tile([P, CH], F16, tag="rsS", name="rsS")
                if dve:
                    nc.vector.tensor_scalar(nmS[:], nmB[:], 1.0, None, OP.mult)
                    nc.vector.tensor_scalar(rsS[:], rsB[:], 1.0, None, OP.mult)
                else:
                    nc.scalar.activation(nmS[:], nmB[:], AF.Copy)
                    nc.scalar.activation(rsS[:], rsB[:], AF.Copy)
                bsp_by_chunk[c] = (nmS, rsS)

            def post_head(c, st):
                # In flush mode (drained inside the next FFN phase) the whole
                # chain runs on DVE — table-free rsqrt via magic-constant
                # seed + 2 Newton steps — so the ACT queue never switches
                # functions mid-Gelu-stream (a switch costs a 1.28us table
                # reload and stalls the PSUM evacuation).  The broadcast
                # matmuls are deferred to the tail queue so the PE reaches
                # them after the DVE chain has finished.
                dve = flush_mode[0]
                nm = smp.tile([1, CH], F16, tag="nm", name="nm")
                t1 = smp.tile([1, CH], F32, tag="t1", name="t1")
                m2 = smp.tile([1, CH], F32, tag="m2", name="m2")
                rsb = smp.tile([1, CH], F16, tag="rsb", name="rsb")
                if dve:
                    nc.vector.tensor_scalar(nm[:], st[0:1, :], -1.0 / E, None,
                                            OP.mult)
                    nc.vector.tensor_scalar(t1[:], st[32:33, :], 1.0 / E, None,
                                            OP.mult)
                    nc.vector.tensor_mul(m2[:], nm[:], nm[:])
                    nc.vector.tensor_sub(t1[:], t1[:], m2[:])      # 256*var
                    nc.vector.tensor_scalar(t1[:], t1[:], EPS * S * S, None,
                                            OP.add)
                    # magic-constant rsqrt seed; the DVE ALU adds in fp32, so
                    # MAGIC - (bits>>1) is computed as a value-domain mul-add
                    # into a u32 tile (values < 2^31, +-64 bit-error is
                    # irrelevant for a 3%-accurate seed)
                    y0u = smp.tile([1, CH], mybir.dt.uint32, tag="y0u",
                                   name="y0u")
                    nc.vector.tensor_scalar(
                        y0u[:], t1[:].bitcast(mybir.dt.uint32), 1, None,
                        OP.logical_shift_right)
                    nc.vector.tensor_scalar(y0u[:], y0u[:], -1.0,
                                            float(0x5f3759df), OP.mult, OP.add)
                    y = y0u[:].bitcast(F32)
                    tt = smp.tile([1, CH], F32, tag="tt", name="tt")
                    for _ in range(2):
                        nc.vector.tensor_mul(tt[:], y, y)
                        nc.vector.tensor_mul(tt[:], tt[:], t1[:])
                        nc.vector.tensor_scalar(tt[:], tt[:], -0.5, 1.5,
                                                OP.mult, OP.add)
                        nc.vector.tensor_mul(y, y, tt[:])
                    nc.vector.tensor_scalar(rsb[:], y, 1.0, None, OP.mult)
                    headq.append(lambda: post_bcast(c, nm, rsb))
                else:
                    nc.scalar.activation(nm[:], st[0:1, :], AF.Copy,
                                         scale=-1.0 / E)
                    nc.scalar.activation(t1[:], st[32:33, :], AF.Copy,
                                         scale=1.0 / E)
                    nc.scalar.activation(m2[:], nm[:], AF.Square)
                    nc.vector.tensor_sub(t1[:], t1[:], m2[:])      # 256*var
                    nc.scalar.activation(t1[:], t1[:], AF.Sqrt, bias=eps_t[:])
                    rs = smp.tile([1, CH], F32, tag="rs", name="rs")
                    nc.vector.reciprocal_approx_fast(out=rs[:], in_=t1[:])
                    nc.scalar.activation(rsb[:], rs[:], AF.Copy)
                    headq.append(lambda: post_bcast(c, nm, rsb))

            def post_tail(c, m):
                cs = slice(c * CH, (c + 1) * CH)
                nmS, rsS = bsp_by_chunk[c]
                xt = resid[m][:, cs]
                tmp = bsp.tile([P, CH], F16, tag="tmp", name="tmp")
                nc.vector.tensor_add(tmp[:], xt, nmS[:])
                nc.vector.tensor_mul(tmp[:], tmp[:], rsS[:])
                o16 = out16t[m][c]
                nc.vector.tensor_scalar(
                    o16[:], tmp[:], g_pb[:, m:m + 1], b_pb[:, m:m + 1],
                    OP.mult, OP.add)
                o8 = out8t[m // 2][:, m % 2, cs]
                if flush_mode[0]:
                    nc.vector.tensor_scalar(o8, o16[:], 1.0 / S, None, OP.mult)
                else:
                    nc.scalar.activation(o8, o16[:], AF.Copy, scale=1.0 / S)

            bsp_by_chunk = {}
            flush_mode = [False]

            def deferred(s):
                if s - 1 >= 0:
                    add_item(s - 1)
                if s - 2 >= 0:
                    sqstat_item(s - 2)
                if headq:
                    headq.pop(0)()
                elif tailq:
                    tailq.pop(0)()
                # 11 deferred items per 8 slots: drain an extra LN tail when
                # the queue backs up so no chunk<=1 fp8 cast leaks into the
                # flush (FFN1 reads chunks 0/1 immediately)
                if len(tailq) > 8:
                    tailq.pop(0)()

            sts = {}
            for s, (c, m) in enumerate(seq):
                if m == 0:
                    sts[c] = stp.tile([64, CH], F32, tag="st", name="st")
                st = sts[c]
                cs = slice(c * CH, (c + 1) * CH)
                if c == 0 and k_outer_c0 and m % 4 == 0:
                    # k-outer over m-quads: start computing as soon as the
                    # first (weight, rhs) k-tile pair lands from DRAM
                    half = range(m, m + 4)
                    pss = {mm: mmp.tile([P, CH], F32, tag="mm", name="mm")
                           for mm in half}
                    for k in range(KT):
                        for mm in half:
                            nc.tensor.matmul(
                                pss[mm][:], lhsT=wt[k][:, mm * P:(mm + 1) * P],
                                rhs=rhs[k][:, cs],
                                start=(k == 0), stop=(k == KT - 1))
                    for mm in half:
                        stash[s + mm - m] = (c, mm, pss[mm], st)
                elif c == 0 and k_outer_c0:
                    pass          # emitted with the quad above
                else:
                    ps = mmp.tile([P, CH], F32, tag="mm", name="mm")
                    for k in range(KT):
                        nc.tensor.matmul(
                            ps[:], lhsT=wt[k][:, m * P:(m + 1) * P],
                            rhs=rhs[k][:, cs],
                            start=(k == 0), stop=(k == KT - 1))
                    stash[s] = (c, m, ps, st)
                deferred(s)

            n = len(seq)

            def set_flush():
                flush_mode[0] = True

            def drainer():
                if headq:
                    headq.pop(0)()
                elif tailq:
                    tailq.pop(0)()

            # head part: the last adds/stats plus the final chunk's DVE-only
            # LN head (table-free rsqrt); tail part: the deferred broadcast
            # matmuls + LN tails, dripped into the next phase's slots.
            backlog = len(tailq)
            head_items = [set_flush,
                          lambda: add_item(n - 1),
                          lambda: sqstat_item(n - 2),
                          lambda: sqstat_item(n - 1),
                          drainer]                      # runs post_head
            tail_items = [drainer] * (3 + backlog + MT)
            return head_items, tail_items

        def ffn_phase(in16, in8, w1q_t, w2q_dram, w2h_dram, b1_pb, b2_pb,
                      residt, out_dram, f2p, pre_flush=()):
            nf16 = HT - 2 * f2p
            """residt[m][:,cs] <- 16*(LN_out + FFN(LN_out)) f16, DMA'd out.

            FFN1 fully fp8 DoubleRow; FFN2 contracts F2P*256 rows fp8 + rest
            f16.  Chunk-pair blocking so every LDWEIGHTS covers 2 matmuls.
            The previous phase's leftover thunks are drip-fed one per slot;
            FFN2 weight m-blocks are DMA-prefetched two m ahead.
            """
            head_items, tail_items = (list(pre_flush[0]), list(pre_flush[1])) \
                if pre_flush else ([], [])

            def pump():
                if tail_items:
                    tail_items.pop(0)()

            def dma_w2(m):
                wq = w2qp.tile([P, 2 * F2PMAX, P], F8, tag="w2q", name="w2q")
                (nc.scalar if m % 2 == 0 else nc.sync).dma_start(
                    out=wq[:, 0:2 * f2p, :], in_=w2q_dram[m * P:(m + 1) * P, :, :])
                wh = w2hp.tile([P, HT - 2 * min(F2PV, F2PN), P], F16,
                               tag="w2h", name="w2h")
                (nc.sync if m % 2 == 0 else nc.scalar).dma_start(
                    out=wh[:, 0:nf16, :], in_=w2h_dram[m * P:(m + 1) * P, :, :])
                return wq, wh

            for cb in range(NCH // 2):
                css = [slice((2 * cb + ci) * CH, (2 * cb + ci + 1) * CH)
                       for ci in range(2)]
                h8 = [[hp.tile([P, 2, CH], F8, tag=f"h8_{kk}_{ci}",
                               name=f"h8_{kk}_{ci}") for ci in range(2)]
                      for kk in range(f2p)]
                h16 = [[hp.tile([P, CH], F16, tag=f"h16_{j}_{ci}",
                                name=f"h16_{j}_{ci}") for ci in range(2)]
                       for j in range(nf16)]
                w2t = {}
                for hm in range(HT):
                    pss = [mmp.tile([P, CH], F32, tag="mm", name="mm")
                           for _ in range(2)]
                    for kk in range(F1P):
                        for ci in range(2):
                            nc.tensor.matmul(
                                pss[ci][:], lhsT=w1q_t[kk][:, :, hm * P:(hm + 1) * P],
                                rhs=in8[kk][:, :, css[ci]],
                                start=(kk == 0), stop=(kk == F1P - 1),
                                perf_mode=DR)
                    for ci in range(2):
                        if hm < 2 * f2p:
                            dst = h8[hm // 2][ci][:, hm % 2, :]
                        else:
                            dst = h16[hm - 2 * f2p][ci][:]
                        nc.scalar.activation(dst, pss[ci][:], AF.Gelu,
                                             bias=b1_pb[:, hm:hm + 1],
                                             scale=1.0 / S)
                    if cb == 0 and hm == 0:
                        # previous phase's deferred adds/stats + its DVE-only
                        # LN head (no ACT-table traffic) drain here
                        for it in head_items:
                            it()
                        head_items = []
                    elif hm >= 5:
                        pump()
                    if hm == HT - 4:
                        w2t[0] = dma_w2(0)
                    elif hm == HT - 2:
                        w2t[1] = dma_w2(1)
                for m in range(MT):
                    if m + 2 < MT:
                        w2t[m + 2] = dma_w2(m + 2)
                    wq, wh = w2t.pop(m)
                    pss = [mmp.tile([P, CH], F32, tag="mm", name="mm")
                           for _ in range(2)]
                    for kk in range(f2p):
                        for ci in range(2):
                            nc.tensor.matmul(
                                pss[ci][:], lhsT=wq[:, 2 * kk:2 * kk + 2, :],
                                rhs=h8[kk][ci][:, :, :],
                                start=(kk == 0),
                                stop=(kk == f2p - 1 and nf16 == 0),
                                perf_mode=DR)
                    for j in range(nf16):
                        for ci in range(2):
                            nc.tensor.matmul(
                                pss[ci][:], lhsT=wh[:, j, :],
                                rhs=h16[j][ci][:],
                                start=(f2p == 0 and j == 0),
                                stop=(j == nf16 - 1))
                    for ci in range(2):
                        ot = residt[m][:, css[ci]]
                        nc.vector.affine_then_add(
                            ot, pss[ci][:], in16[m][2 * cb + ci][:],
                            scale=1.0, bias=b2_pb[:, m:m + 1])
                        (nc.sync if ci == 0 else nc.scalar).dma_start(
                            out=out_dram[m * P:(m + 1) * P, css[ci]], in_=ot)
                    pump()
            for it in head_items:
                it()
            while tail_items:
                tail_items.pop(0)()

        _REP = int(os.environ.get("BENCH_REPEAT", "1"))
        for _rep in range(_REP):
            # ---- phase A: verb attends to noun, LN -> verb1 ----
            # DMA order follows first-chunk consumption: the c0 m-quad only
            # needs wa[k][:,0:512] + nt[k][:,0:512] per k, so those stream
            # first (sync + vector queues: the scalar queue is blocked for
            # ~11.5us by the startup ACT table loads), then the second wa
            # half (quad 2), then vt chunk 0 (chunk-0 epilogues), then the
            # bulk.
            nt_t, ntb = full_tiles(rhsp, "n")    # 16*(noun+c2); phase-C residual
            vt_t, vtb = full_tiles(resp, "v")    # 16*(verb + c1 - W1@c2)
            wab = watp.tile([P, KT, E], F16, tag="wabig", name="wabig")
            wa1 = [wab[:, k, :] for k in range(KT)]
            # critical stream feeding the first chunk's k-outer quads:
            # per-k descriptors on two queues, everything else batched
            for k in range(KT):
                nc.sync.dma_start(out=wab[:, k, 0:CH], in_=wvo1[:, k, 0:CH])
                nc.gpsimd.dma_start(out=ntb[:, k, 0:CH], in_=nT[:, k, 0:CH])
            nc.sync.dma_start(out=wab[:, :, CH:E], in_=wvo1[:, :, CH:E])
            nc.gpsimd.dma_start(out=vtb[:, :, 0:CH], in_=vT[:, :, 0:CH])
            lnv_pb = load_const(lnv, [P, 2 * MT], "lnv")
            lnn_pb = load_const(lnn, [P, 2 * MT], "lnn")
            b1v_pb = load_const(b1v, [P, HT], "b1v")
            b2v_pb = load_const(b2v, [P, MT], "b2v")
            b1n_pb = load_const(b1n, [P, HT], "b1n")
            b2n_pb = load_const(b2n, [P, MT], "b2n")
            nc.sync.dma_start(out=ntb[:, :, CH:B], in_=nT[:, :, CH:B])
            nc.gpsimd.dma_start(out=vtb[:, :, CH:B], in_=vT[:, :, CH:B])
            w1v_t = load_w1q(w1qv)               # prefetch for phase B
            verb1 = chunk_tiles(lnp, "l")
            v1q = pair_tiles(l8p, "q")
            fl_a = attn_phase(wa1, nt_t, vt_t, verb1, v1q, lnv_pb,
                              k_outer_c0=True)

            # ---- phase B: verb FFN -> verb2' (written into the vT tiles) ----
            wa2 = load_wat(wvo2)                 # prefetch for phase C
            ffn_phase(verb1, v1q, w1v_t, w2qv, w2hv, b1v_pb, b2v_pb,
                      vt_t, verb_out, F2PV, pre_flush=fl_a)

            # ---- phase C: noun attends to verb2, LN -> noun1 ----
            w1n_t = load_w1q(w1qn)               # prefetch for phase D
            noun1 = chunk_tiles(lnp, "l")
            n1q = pair_tiles(l8p, "q")
            fl_c = attn_phase(wa2, vt_t, nt_t, noun1, n1q, lnn_pb)

            # ---- phase D: noun FFN -> noun2' (written into the nT tiles) ----
            ffn_phase(noun1, n1q, w1n_t, w2qn, w2hn, b1n_pb, b2n_pb,
                      nt_t, noun_out, F2PN, pre_flush=fl_c)

    nc.finalize()
    return nc


_prog_cache = {}


def _get_program():
    if "nc" not in _prog_cache:
        _prog_cache["nc"] = _build_program()
    return _prog_cache["nc"]


def _pvec(v, ntiles):
    # [ntiles*128] -> [128, ntiles] with (p, t) = v[t*128+p]
    return np.ascontiguousarray(np.asarray(v, np.float32).reshape(ntiles, P).T)


def _prepare_maps(inputs):
    f32 = np.float32
    f16 = np.float16
    f8 = ml_dtypes.float8_e4m3fn
    g = {k: np.asarray(v, f32) for k, v in inputs.items()}

    def fold(p):
        w = g[f"{p}_wo"] @ g[f"{p}_wv"]
        b = g[f"{p}_wo"] @ g[f"{p}_bv"] + g[f"{p}_bo"]
        return w, b

    def w1pack(w1):
        # [F1P*128, 2, H2]; [kk*128+p, i, h] = fp8(S * w1[h, (2kk+i)*128+p])
        w1T = np.ascontiguousarray(S * w1.T)  # [E, H2]
        r = w1T.reshape(F1P, 2, P, H2).transpose(0, 2, 1, 3)
        return np.ascontiguousarray(r.reshape(F1P * P, 2, H2)).astype(f8)

    def w2pack(w2, f2p):
        # fp8 part [MT*128, 2*f2p, 128]:
        #   [m*128+p, 2kk+i, mc] = fp8(S * w2[m*128+mc, (2kk+i)*128+p])
        # f16 part [MT*128, nf16, 128]:
        #   [m*128+p, j, mc] = f16(S * w2[m*128+mc, (2*f2p+j)*128+p])
        nf16 = HT - 2 * f2p
        w2s = S * w2  # [E, H2]
        r = w2s.reshape(MT, P, HT, P)         # [m, mc, ht, p]
        r = r.transpose(0, 3, 2, 1)           # [m, p, ht, mc]
        q = np.ascontiguousarray(
            r[:, :, :2 * f2p, :].reshape(MT * P, 2 * f2p, P)).astype(f8)
        h = np.ascontiguousarray(
            r[:, :, 2 * f2p:, :].reshape(MT * P, nf16, P)).astype(f16)
        return q, h

    W1f, c1 = fold("v2n")
    W2f, c2 = fold("n2v")
    c_fix = W1f @ c2
    w2qv_a, w2hv_a = w2pack(g["fv_w2"], F2PV)
    w2qn_a, w2hn_a = w2pack(g["fn_w2"], F2PN)
    def kmaj(w):
        # [E, cols] -> [P, KT, cols]
        c = w.shape[1]
        return np.ascontiguousarray(
            w.reshape(KT, P, c).transpose(1, 0, 2))

    common = {
        "wvo1": kmaj(np.ascontiguousarray(W1f.T)).astype(f16),
        "wvo2": kmaj(np.ascontiguousarray(W2f.T)).astype(f16),
        "lnv": np.concatenate([_pvec(S * g["ln_v_g"], MT),
                               _pvec(S * g["ln_v_b"], MT)], axis=1),
        "lnn": np.concatenate([_pvec(S * g["ln_n_g"], MT),
                               _pvec(S * g["ln_n_b"], MT)], axis=1),
        "w1qv": w1pack(g["fv_w1"]), "b1v": _pvec(g["fv_b1"], HT),
        "w1qn": w1pack(g["fn_w1"]), "b1n": _pvec(g["fn_b1"], HT),
        "w2qv": w2qv_a, "w2hv": w2hv_a, "b2v": _pvec(S * g["fv_b2"], MT),
        "w2qn": w2qn_a, "w2hn": w2hn_a, "b2n": _pvec(S * g["fn_b2"], MT),
    }
    vT = kmaj(S * (g["verb_features"].T + (c1 - c_fix).reshape(E, 1))).astype(f16)
    nT = kmaj(S * (g["noun_features"].T + c2.reshape(E, 1))).astype(f16)
    in_maps = []
    for i in range(NCORES):
        cs = slice(i * B, (i + 1) * B)
        m = dict(common)
        m["vT"] = np.ascontiguousarray(vT[:, :, cs])
        m["nT"] = np.ascontiguousarray(nT[:, :, cs])
        in_maps.append(m)
    return in_maps


def kernel(**inputs):
    nc = _get_program()
    in_maps = _prepare_maps(inputs)
    res = run_bass_kernel_spmd(nc, in_maps, list(range(NCORES))).results
    verb = np.concatenate(
        [res[i]["verb_out"].astype(np.float32) for i in range(NCORES)], axis=1)
    noun = np.concatenate(
        [res[i]["noun_out"].astype(np.float32) for i in range(NCORES)], axis=1)
    return (np.ascontiguousarray(verb.T) * np.float32(1.0 / S),
            np.ascontiguousarray(noun.T) * np.float32(1.0 / S))


# revision 44
# speedup vs baseline: 1.0050x; 1.0050x over previous
"""Trainium2 Bass kernel for a dense cross-task transformer block (v2: fp8).

Math notes
----------
The reference "attention" has sequence length 1 on the key axis, so
softmax(scores) == 1.0 exactly and the whole q/k/score path is dead:

    mha_len1(q_in, kv_in, ...) == (kv_in @ wv.T + bv) @ wo.T + bo

which folds (on host) into a single matmul with W = wo @ wv and
b = wo @ bv + bo.  The block is then:

    verb1 = LN(verb + noun @ W1.T + c1)          (ln_v)
    verb2 = verb1 + FFN_v(verb1)
    noun1 = LN(noun + verb2 @ W2.T + c2)         (ln_n)
    noun2 = noun1 + FFN_n(noun1)
    return verb2, noun2

Bias folding: nT is pre-loaded as 16*(noun + c2) so it serves both as the
phase-C residual (which needs +c2) and the phase-A rhs; the resulting
spurious W1@c2 term in phase A is cancelled by pre-subtracting W1@c2 from
the verb residual on the host.  (All biases are zero in practice.)

Precision scheme (the kernel is PE-issue-bound, so FLOP dtype = speed):
 - Everything stored/streamed in fp16 (not bf16: 8x less rounding noise),
   activations and residuals carried at 16x scale so fp8-weight matmuls,
   f16-weight matmuls and residual adds all share one PSUM scale.  LN is
   scale-invariant, outputs are divided by 16 on the host.
 - FFN1 runs fully in fp8 e4m3 DoubleRow (2 contraction rows/cycle):
   lhsT = fp8(16*w1) pairs, rhs = fp8(LN output, true scale).
 - FFN2 contracts 2048: first F2P*256 rows in fp8 DoubleRow (gelu evacuates
   those h-tiles straight to fp8 pair tiles), the rest in f16.
 - Attention GEMMs stay f16 (their quantization noise is amplified ~2x
   through the LN + downstream path; FFN noise is the cheapest to spend).
   The fp8/f16 mix is chosen so total rel-err ~1.7e-2 < 2e-2 budget.

Engine choreography:
 - attention epilogues (PSUM evac, x^2, LN stats matmuls) are emitted one
   m-tile late so the PE never waits on DVE/ACT; stats PSUM is
   double-buffered across chunks (sx at partition 0, sqs at partition 32
   of one bank) so chunk c+1 never WARs on chunk c's stats reads.
 - LN mean/rstd broadcast via rank-1 matmuls, copied to f16 SBUF; the
   per-tile LN tail is 3 DVE ops + 1 ACT fp8 cast.
 - FFN weights stream per m-block; FFN1/FFN2 process chunk-pairs so each
   DoubleRow LDWEIGHTS (213ns, no FWL) amortizes over 2 matmuls.
"""

import os
import numpy as np
import ml_dtypes
from contextlib import ExitStack

import concourse.bass as bass
import concourse.bacc as bacc_mod
import concourse.mybir as mybir
import concourse.tile as tile
from concourse.bass_utils import run_bass_kernel_spmd

E = 1024          # embed dim
H2 = 2048         # FFN hidden dim
B_TOTAL = 16384
NCORES = 8
B = B_TOTAL // NCORES   # 2048 rows per core
P = 128
EPS = 1e-5
CH = 512          # column chunk
NCH = B // CH     # 4
KT = E // P       # 8
MT = E // P       # 8
HT = H2 // P      # 16
S = 16.0          # global activation/residual scale
F1P = 4           # FFN1 fp8 pairs (of KT//2=4): fully fp8
F2PV = 6          # verb FFN2 fp8 pairs (of HT//2=8)
F2PN = 5          # noun FFN2 fp8 pairs
F2PMAX = max(F2PV, F2PN)

F32 = mybir.dt.float32
F16 = mybir.dt.float16
F8 = mybir.dt.float8e4
AF = mybir.ActivationFunctionType
OP = mybir.AluOpType
DR = mybir.MatmulPerfMode.DoubleRow


def _build_program():
    nc = bacc_mod.Bacc("TRN2", target_bir_lowering=False)

    # activations/attention weights laid out [P, KT, cols] so bulk loads can
    # be single multi-k-tile DMA descriptors (startup is descriptor-bound)
    vT = nc.declare_dram_parameter("vT", [P, KT, B], F16, isOutput=False)
    nT = nc.declare_dram_parameter("nT", [P, KT, B], F16, isOutput=False)
    wvo1 = nc.declare_dram_parameter("wvo1", [P, KT, E], F16, isOutput=False)
    wvo2 = nc.declare_dram_parameter("wvo2", [P, KT, E], F16, isOutput=False)
    lnv = nc.declare_dram_parameter("lnv", [P, 2 * MT], F32, isOutput=False)  # 16g|16b
    lnn = nc.declare_dram_parameter("lnn", [P, 2 * MT], F32, isOutput=False)
    w1qv = nc.declare_dram_parameter("w1qv", [F1P * P, 2, H2], F8, isOutput=False)
    w1qn = nc.declare_dram_parameter("w1qn", [F1P * P, 2, H2], F8, isOutput=False)
    b1v = nc.declare_dram_parameter("b1v", [P, HT], F32, isOutput=False)
    b1n = nc.declare_dram_parameter("b1n", [P, HT], F32, isOutput=False)
    w2qv = nc.declare_dram_parameter("w2qv", [MT * P, 2 * F2PV, P], F8, isOutput=False)
    w2qn = nc.declare_dram_parameter("w2qn", [MT * P, 2 * F2PN, P], F8, isOutput=False)
    w2hv = nc.declare_dram_parameter("w2hv", [MT * P, HT - 2 * F2PV, P], F16,
                                     isOutput=False)
    w2hn = nc.declare_dram_parameter("w2hn", [MT * P, HT - 2 * F2PN, P], F16,
                                     isOutput=False)
    b2v = nc.declare_dram_parameter("b2v", [P, MT], F32, isOutput=False)  # 16*b2
    b2n = nc.declare_dram_parameter("b2n", [P, MT], F32, isOutput=False)
    verb_out = nc.declare_dram_parameter("verb_out", [E, B], F16, isOutput=True)
    noun_out = nc.declare_dram_parameter("noun_out", [E, B], F16, isOutput=True)

    with tile.TileContext(nc) as tc, ExitStack() as ctx:
        const = ctx.enter_context(tc.tile_pool(name="const", bufs=1))
        rhsp = ctx.enter_context(tc.tile_pool(name="rhsp", bufs=1))    # nT; later noun2'
        resp = ctx.enter_context(tc.tile_pool(name="resp", bufs=1))    # vT; later verb2'
        lnp = ctx.enter_context(tc.tile_pool(name="lnp", bufs=1))      # verb1'/noun1' f16
        l8p = ctx.enter_context(tc.tile_pool(name="l8p", bufs=1))      # verb1/noun1 fp8 pairs
        watp = ctx.enter_context(tc.tile_pool(name="watp", bufs=1))    # wvo1/wvo2
        w1p = ctx.enter_context(tc.tile_pool(name="w1p", bufs=1))      # w1 fp8 pair tiles
        w2qp = ctx.enter_context(tc.tile_pool(name="w2qp", bufs=3))    # w2 fp8 m-block stream
        w2hp = ctx.enter_context(tc.tile_pool(name="w2hp", bufs=3))    # w2 f16 m-block stream
        hp = ctx.enter_context(tc.tile_pool(name="hp", bufs=1))        # gelu hidden
        sqp = ctx.enter_context(tc.tile_pool(name="sqp", bufs=1))      # x^2 (8 tags)
        smp = ctx.enter_context(tc.tile_pool(name="smp", bufs=1))      # LN stats vectors
        bsp = ctx.enter_context(tc.tile_pool(name="bsp", bufs=3))      # bcast SBUF copies
        mmp = ctx.enter_context(tc.tile_pool(name="mmp", bufs=4, space="PSUM"))
        stp = ctx.enter_context(tc.tile_pool(name="stp", bufs=2, space="PSUM"))
        bcp = ctx.enter_context(tc.tile_pool(name="bcp", bufs=1, space="PSUM"))

        ones_col = const.tile([P, 1], F16, tag="ones_col", name="ones_col")
        nc.vector.memset(ones_col[:], 1.0)
        ones_row = const.tile([1, P], F16, tag="ones_row", name="ones_row")
        nc.vector.memset(ones_row[:], 1.0)
        eps_t = const.tile([1, 1], F32, tag="eps", name="eps")
        nc.vector.memset(eps_t[:], EPS * S * S)

        def load_const(dram, shape, tag, dtype=F32):
            t = const.tile(shape, dtype, tag=tag, name=tag)
            nc.scalar.dma_start(out=t[:], in_=dram[:, :])
            return t

        def chunk_tiles(pool, pfx):
            return [[pool.tile([P, CH], F16, tag=f"{pfx}{k}c{c}", name=f"{pfx}{k}c{c}")
                     for c in range(NCH)] for k in range(KT)]

        def pair_tiles(pool, pfx):
            return [pool.tile([P, 2, B], F8, tag=f"{pfx}{kk}", name=f"{pfx}{kk}")
                    for kk in range(KT // 2)]

        def full_tiles(pool, pfx):
            # one big tile so bulk loads can be single multi-k DMA descriptors
            big = pool.tile([P, KT, B], F16, tag=f"{pfx}big", name=f"{pfx}big")
            return [big[:, k, :] for k in range(KT)], big

        def load_wat(dram):
            big = watp.tile([P, KT, E], F16, tag="wabig", name="wabig")
            half = KT // 2
            nc.scalar.dma_start(out=big[:, 0:half, :], in_=dram[:, 0:half, :])
            nc.sync.dma_start(out=big[:, half:KT, :], in_=dram[:, half:KT, :])
            return [big[:, k, :] for k in range(KT)]

        def load_w1q(dram):
            tiles = []
            for kk in range(F1P):
                t = w1p.tile([P, 2, H2], F8, tag=f"w1_{kk}", name=f"w1_{kk}")
                eng = nc.scalar if kk % 2 == 0 else nc.sync
                eng.dma_start(out=t[:, :, :], in_=dram[kk * P:(kk + 1) * P, :, :])
                tiles.append(t)
            return tiles

        def attn_phase(wt, rhs, resid, out16t, out8t, gb_pb, k_outer_c0=False):
            """resid[m][:,cs] <- xt' = resid + wt.T@rhs (16x, f16, in place);
            out16t[m][c] <- 16*LN(x) f16; out8t[kk][:,m%2,cs] <- fp8 LN(x).

            Pipelined epilogues: the PSUM-evac add trails its matmul group by
            one m-slot, the x^2 + stats matmuls by two (so the PE's in-order
            queue never waits on DVE/ACT), and the per-chunk LN post work is
            sliced per-m and drip-fed one item per slot.  Unfinished items are
            returned as a thunk list the next phase interleaves into itself.
            """
            g_pb = gb_pb[:, 0:MT]
            b_pb = gb_pb[:, MT:2 * MT]
            seq = [(c, m) for c in range(NCH) for m in range(MT)]
            stash = {}
            headq = []
            tailq = []

            def add_item(s):
                c, m, ps, st = stash[s]
                cs = slice(c * CH, (c + 1) * CH)
                xt = resid[m][:, cs]
                nc.vector.tensor_add(xt, ps[:], xt)

            def sqstat_item(s):
                # x^2 per m-slot (spread over the ACT/DVE queues); the 16
                # stats matmuls are emitted as one per-chunk batch so the PE
                # pays the array-reconfig hiccup twice per chunk, not 16x.
                c, m, ps, st = stash.pop(s)
                cs = slice(c * CH, (c + 1) * CH)
                xt = resid[m][:, cs]
                sq = sqp.tile([P, CH], F16, tag=f"sq{m}", name=f"sq{m}")
                sqt[m] = sq
                if flush_mode[0]:
                    nc.vector.tensor_mul(sq[:], xt, xt)
                else:
                    nc.scalar.activation(sq[:], xt, AF.Square)
                if m == MT - 1:
                    headq.insert(0, lambda: stats_batch(c, st))

            def stats_batch(c, st):
                cs = slice(c * CH, (c + 1) * CH)
                for m in range(MT):
                    nc.tensor.matmul(st[0:1, :], lhsT=ones_col[:],
                                     rhs=resid[m][:, cs],
                                     start=(m == 0), stop=(m == MT - 1))
                    nc.tensor.matmul(st[32:33, :], lhsT=ones_col[:],
                                     rhs=sqt[m][:],
                                     start=(m == 0), stop=(m == MT - 1))
                headq.append(lambda: post_head(c, st))
                for mm in range(MT):
                    tailq.append(lambda c=c, mm=mm: post_tail(c, mm))

            sqt = {}

            def post_bcast(c, nm, rsb):
                dve = flush_mode[0]
                nmB = bcp.tile([P, CH], F32, tag="nmB", name="nmB")
                nc.tensor.matmul(nmB[:], lhsT=ones_row[:], rhs=nm[:],
                                 start=True, stop=True)
                rsB = bcp.tile([P, CH], F32, tag="rsB", name="rsB")
                nc.tensor.matmul(rsB[:], lhsT=ones_row[:], rhs=rsb[:],
                                 start=True, stop=True)
                nmS = bsp.tile([P, CH], F16, tag="nmS", name="nmS")
                rsS = bsp.tile([P, CH], F16, tag="rsS", name="rsS")
                if dve:
                    nc.vector.tensor_scalar(nmS[:], nmB[:], 1.0, None, OP.mult)
                    nc.vector.tensor_scalar(rsS[:], rsB[:], 1.0, None, OP.mult)
                else:
                    nc.scalar.activation(nmS[:], nmB[:], AF.Copy)
                    nc.scalar.activation(rsS[:], rsB[:], AF.Copy)
                bsp_by_chunk[c] = (nmS, rsS)

            def post_head(c, st):
                # In flush mode (drained inside the next FFN phase) the whole
                # chain runs on DVE — table-free rsqrt via magic-constant
                # seed + 2 Newton steps — so the ACT queue never switches
                # functions mid-Gelu-stream (a switch costs a 1.28us table
                # reload and stalls the PSUM evacuation).  The broadcast
                # matmuls are deferred to the tail queue so the PE reaches
                # them after the DVE chain has finished.
                dve = flush_mode[0]
                nm = smp.tile([1, CH], F16, tag="nm", name="nm")
                t1 = smp.tile([1, CH], F32, tag="t1", name="t1")
                m2 = smp.tile([1, CH], F32, tag="m2", name="m2")
                rsb = smp.tile([1, CH], F16, tag="rsb", name="rsb")
                if dve:
                    nc.vector.tensor_scalar(nm[:], st[0:1, :], -1.0 / E, None,
                                            OP.mult)
                    nc.vector.tensor_scalar(t1[:], st[32:33, :], 1.0 / E, None,
                                            OP.mult)
                    nc.vector.tensor_mul(m2[:], nm[:], nm[:])
                    nc.vector.tensor_sub(t1[:], t1[:], m2[:])      # 256*var
                    nc.vector.tensor_scalar(t1[:], t1[:], EPS * S * S, None,
                                            OP.add)
                    # magic-constant rsqrt seed; the DVE ALU adds in fp32, so
                    # MAGIC - (bits>>1) is computed as a value-domain mul-add
                    # into a u32 tile (values < 2^31, +-64 bit-error is
                    # irrelevant for a 3%-accurate seed)
                    y0u = smp.tile([1, CH], mybir.dt.uint32, tag="y0u",
                                   name="y0u")
                    nc.vector.tensor_scalar(
                        y0u[:], t1[:].bitcast(mybir.dt.uint32), 1, None,
                        OP.logical_shift_right)
                    nc.vector.tensor_scalar(y0u[:], y0u[:], -1.0,
                                            float(0x5f3759df), OP.mult, OP.add)
                    y = y0u[:].bitcast(F32)
                    tt = smp.tile([1, CH], F32, tag="tt", name="tt")
                    for _ in range(2):
                        nc.vector.tensor_mul(tt[:], y, y)
                        nc.vector.tensor_mul(tt[:], tt[:], t1[:])
                        nc.vector.tensor_scalar(tt[:], tt[:], -0.5, 1.5,
                                                OP.mult, OP.add)
                        nc.vector.tensor_mul(y, y, tt[:])
                    nc.vector.tensor_scalar(rsb[:], y, 1.0, None, OP.mult)
                    headq.append(lambda: post_bcast(c, nm, rsb))
                else:
                    nc.scalar.activation(nm[:], st[0:1, :], AF.Copy,
                                         scale=-1.0 / E)
                    nc.scalar.activation(t1[:], st[32:33, :], AF.Copy,
                                         scale=1.0 / E)
                    nc.scalar.activation(m2[:], nm[:], AF.Square)
                    nc.vector.tensor_sub(t1[:], t1[:], m2[:])      # 256*var
                    nc.scalar.activation(t1[:], t1[:], AF.Sqrt, bias=eps_t[:])
                    rs = smp.tile([1, CH], F32, tag="rs", name="rs")
                    nc.vector.reciprocal_approx_fast(out=rs[:], in_=t1[:])
                    nc.scalar.activation(rsb[:], rs[:], AF.Copy)
                    headq.append(lambda: post_bcast(c, nm, rsb))

            def post_tail(c, m):
                cs = slice(c * CH, (c + 1) * CH)
                nmS, rsS = bsp_by_chunk[c]
                xt = resid[m][:, cs]
                tmp = bsp.tile([P, CH], F16, tag="tmp", name="tmp")
                nc.vector.tensor_add(tmp[:], xt, nmS[:])
                nc.vector.tensor_mul(tmp[:], tmp[:], rsS[:])
                o16 = out16t[m][c]
                nc.vector.tensor_scalar(
                    o16[:], tmp[:], g_pb[:, m:m + 1], b_pb[:, m:m + 1],
                    OP.mult, OP.add)
                o8 = out8t[m // 2][:, m % 2, cs]
                if flush_mode[0]:
                    nc.vector.tensor_scalar(o8, o16[:], 1.0 / S, None, OP.mult)
                else:
                    nc.scalar.activation(o8, o16[:], AF.Copy, scale=1.0 / S)

            bsp_by_chunk = {}
            flush_mode = [False]

            def deferred(s):
                if s - 1 >= 0:
                    add_item(s - 1)
                if s - 2 >= 0:
                    sqstat_item(s - 2)
                if headq:
                    headq.pop(0)()
                elif tailq:
                    tailq.pop(0)()
                # 11 deferred items per 8 slots: drain an extra LN tail when
                # the queue backs up so no chunk<=1 fp8 cast leaks into the
                # flush (FFN1 reads chunks 0/1 immediately)
                if len(tailq) > 8:
                    tailq.pop(0)()

            sts = {}
            for s, (c, m) in enumerate(seq):
                if m == 0:
                    sts[c] = stp.tile([64, CH], F32, tag="st", name="st")
                st = sts[c]
                cs = slice(c * CH, (c + 1) * CH)
                if c == 0 and k_outer_c0 and m % 4 == 0:
                    # k-outer over m-quads: start computing as soon as the
                    # first (weight, rhs) k-tile pair lands from DRAM
                    half = range(m, m + 4)
                    pss = {mm: mmp.tile([P, CH], F32, tag="mm", name="mm")
                           for mm in half}
                    for k in range(KT):
                        for mm in half:
                            nc.tensor.matmul(
                                pss[mm][:], lhsT=wt[k][:, mm * P:(mm + 1) * P],
                                rhs=rhs[k][:, cs],
                                start=(k == 0), stop=(k == KT - 1))
                    for mm in half:
                        stash[s + mm - m] = (c, mm, pss[mm], st)
                elif c == 0 and k_outer_c0:
                    pass          # emitted with the quad above
                else:
                    ps = mmp.tile([P, CH], F32, tag="mm", name="mm")
                    for k in range(KT):
                        nc.tensor.matmul(
                            ps[:], lhsT=wt[k][:, m * P:(m + 1) * P],
                            rhs=rhs[k][:, cs],
                            start=(k == 0), stop=(k == KT - 1))
                    stash[s] = (c, m, ps, st)
                deferred(s)

            n = len(seq)

            def set_flush():
                flush_mode[0] = True

            def drainer():
                if headq:
                    headq.pop(0)()
                elif tailq:
                    tailq.pop(0)()

            # head part: the last adds/stats plus the final chunk's DVE-only
            # LN head (table-free rsqrt); tail part: the deferred broadcast
            # matmuls + LN tails, dripped into the next phase's slots.
            backlog = len(tailq)
            head_items = [set_flush,
                          lambda: add_item(n - 1),
                          lambda: sqstat_item(n - 2),
                          lambda: sqstat_item(n - 1),
                          drainer]                      # runs post_head
            tail_items = [drainer] * (3 + backlog + MT)
            return head_items, tail_items

        def ffn_phase(in16, in8, w1q_t, w2q_dram, w2h_dram, b1_pb, b2_pb,
                      residt, out_dram, f2p, pre_flush=()):
            nf16 = HT - 2 * f2p
            """residt[m][:,cs] <- 16*(LN_out + FFN(LN_out)) f16, DMA'd out.

            FFN1 fully fp8 DoubleRow; FFN2 contracts F2P*256 rows fp8 + rest
            f16.  Chunk-pair blocking so every LDWEIGHTS covers 2 matmuls.
            The previous phase's leftover thunks are drip-fed one per slot;
            FFN2 weight m-blocks are DMA-prefetched two m ahead.
            """
            head_items, tail_items = (list(pre_flush[0]), list(pre_flush[1])) \
                if pre_flush else ([], [])

            def pump():
                if tail_items:
                    tail_items.pop(0)()

            def dma_w2(m):
                wq = w2qp.tile([P, 2 * F2PMAX, P], F8, tag="w2q", name="w2q")
                (nc.scalar if m % 2 == 0 else nc.sync).dma_start(
                    out=wq[:, 0:2 * f2p, :], in_=w2q_dram[m * P:(m + 1) * P, :, :])
                wh = w2hp.tile([P, HT - 2 * min(F2PV, F2PN), P], F16,
                               tag="w2h", name="w2h")
                (nc.sync if m % 2 == 0 else nc.scalar).dma_start(
                    out=wh[:, 0:nf16, :], in_=w2h_dram[m * P:(m + 1) * P, :, :])
                return wq, wh

            for cb in range(NCH // 2):
                css = [slice((2 * cb + ci) * CH, (2 * cb + ci + 1) * CH)
                       for ci in range(2)]
                h8 = [[hp.tile([P, 2, CH], F8, tag=f"h8_{kk}_{ci}",
                               name=f"h8_{kk}_{ci}") for ci in range(2)]
                      for kk in range(f2p)]
                h16 = [[hp.tile([P, CH], F16, tag=f"h16_{j}_{ci}",
                                name=f"h16_{j}_{ci}") for ci in range(2)]
                       for j in range(nf16)]
                w2t = {}
                for hm in range(HT):
                    pss = [mmp.tile([P, CH], F32, tag="mm", name="mm")
                           for _ in range(2)]
                    for kk in range(F1P):
                        for ci in range(2):
                            nc.tensor.matmul(
                                pss[ci][:], lhsT=w1q_t[kk][:, :, hm * P:(hm + 1) * P],
                                rhs=in8[kk][:, :, css[ci]],
                                start=(kk == 0), stop=(kk == F1P - 1),
                                perf_mode=DR)
                    for ci in range(2):
                        if hm < 2 * f2p:
                            dst = h8[hm // 2][ci][:, hm % 2, :]
                        else:
                            dst = h16[hm - 2 * f2p][ci][:]
                        nc.scalar.activation(dst, pss[ci][:], AF.Gelu,
                                             bias=b1_pb[:, hm:hm + 1],
                                             scale=1.0 / S)
                    if cb == 0 and hm == 0:
                        # previous phase's deferred adds/stats + its DVE-only
                        # LN head (no ACT-table traffic) drain here
                        for it in head_items:
                            it()
                        head_items = []
                    elif hm >= 5:
                        pump()
                    if hm == HT - 4:
                        w2t[0] = dma_w2(0)
                    elif hm == HT - 2:
                        w2t[1] = dma_w2(1)
                for m in range(MT):
                    if m + 2 < MT:
                        w2t[m + 2] = dma_w2(m + 2)
                    wq, wh = w2t.pop(m)
                    pss = [mmp.tile([P, CH], F32, tag="mm", name="mm")
                           for _ in range(2)]
                    for kk in range(f2p):
                        for ci in range(2):
                            nc.tensor.matmul(
                                pss[ci][:], lhsT=wq[:, 2 * kk:2 * kk + 2, :],
                                rhs=h8[kk][ci][:, :, :],
                                start=(kk == 0),
                                stop=(kk == f2p - 1 and nf16 == 0),
                                perf_mode=DR)
                    for j in range(nf16):
                        for ci in range(2):
                            nc.tensor.matmul(
                                pss[ci][:], lhsT=wh[:, j, :],
                                rhs=h16[j][ci][:],
                                start=(f2p == 0 and j == 0),
                                stop=(j == nf16 - 1))
                    for ci in range(2):
                        ot = residt[m][:, css[ci]]
                        nc.vector.affine_then_add(
                            ot, pss[ci][:], in16[m][2 * cb + ci][:],
                            scale=1.0, bias=b2_pb[:, m:m + 1])
                        (nc.sync if ci == 0 else nc.scalar).dma_start(
                            out=out_dram[m * P:(m + 1) * P, css[ci]], in_=ot)
                    pump()
            for it in head_items:
                it()
            while tail_items:
                tail_items.pop(0)()

        _REP = int(os.environ.get("BENCH_REPEAT", "1"))
        for _rep in range(_REP):
            # ---- phase A: verb attends to noun, LN -> verb1 ----
            # DMA order follows first-chunk consumption: the c0 m-quad only
            # needs wa[k][:,0:512] + nt[k][:,0:512] per k, so those stream
            # first (sync + vector queues: the scalar queue is blocked for
            # ~11.5us by the startup ACT table loads), then the second wa
            # half (quad 2), then vt chunk 0 (chunk-0 epilogues), then the
            # bulk.
            nt_t, ntb = full_tiles(rhsp, "n")    # 16*(noun+c2); phase-C residual
            vt_t, vtb = full_tiles(resp, "v")    # 16*(verb + c1 - W1@c2)
            wab = watp.tile([P, KT, E], F16, tag="wabig", name="wabig")
            wa1 = [wab[:, k, :] for k in range(KT)]
            # critical stream feeding the first chunk's k-outer quads:
            # per-k descriptors on two queues, everything else batched
            for k in range(KT):
                nc.sync.dma_start(out=wab[:, k, 0:CH], in_=wvo1[:, k, 0:CH])
                nc.gpsimd.dma_start(out=ntb[:, k, 0:CH], in_=nT[:, k, 0:CH])
            nc.sync.dma_start(out=wab[:, 0:4, CH:E], in_=wvo1[:, 0:4, CH:E])
            nc.sync.dma_start(out=wab[:, 4:KT, CH:E], in_=wvo1[:, 4:KT, CH:E])
            nc.gpsimd.dma_start(out=vtb[:, :, 0:CH], in_=vT[:, :, 0:CH])
            lnv_pb = load_const(lnv, [P, 2 * MT], "lnv")
            lnn_pb = load_const(lnn, [P, 2 * MT], "lnn")
            b1v_pb = load_const(b1v, [P, HT], "b1v")
            b2v_pb = load_const(b2v, [P, MT], "b2v")
            b1n_pb = load_const(b1n, [P, HT], "b1n")
            b2n_pb = load_const(b2n, [P, MT], "b2n")
            for c in range(1, NCH):
                cs = slice(c * CH, (c + 1) * CH)
                nc.sync.dma_start(out=ntb[:, :, cs], in_=nT[:, :, cs])
                nc.gpsimd.dma_start(out=vtb[:, :, cs], in_=vT[:, :, cs])
            w1v_t = load_w1q(w1qv)               # prefetch for phase B
            verb1 = chunk_tiles(lnp, "l")
            v1q = pair_tiles(l8p, "q")
            fl_a = attn_phase(wa1, nt_t, vt_t, verb1, v1q, lnv_pb,
                              k_outer_c0=True)

            # ---- phase B: verb FFN -> verb2' (written into the vT tiles) ----
            wa2 = load_wat(wvo2)                 # prefetch for phase C
            ffn_phase(verb1, v1q, w1v_t, w2qv, w2hv, b1v_pb, b2v_pb,
                      vt_t, verb_out, F2PV, pre_flush=fl_a)

            # ---- phase C: noun attends to verb2, LN -> noun1 ----
            w1n_t = load_w1q(w1qn)               # prefetch for phase D
            noun1 = chunk_tiles(lnp, "l")
            n1q = pair_tiles(l8p, "q")
            fl_c = attn_phase(wa2, vt_t, nt_t, noun1, n1q, lnn_pb)

            # ---- phase D: noun FFN -> noun2' (written into the nT tiles) ----
            ffn_phase(noun1, n1q, w1n_t, w2qn, w2hn, b1n_pb, b2n_pb,
                      nt_t, noun_out, F2PN, pre_flush=fl_c)

    nc.finalize()
    return nc


_prog_cache = {}


def _get_program():
    if "nc" not in _prog_cache:
        _prog_cache["nc"] = _build_program()
    return _prog_cache["nc"]


def _pvec(v, ntiles):
    # [ntiles*128] -> [128, ntiles] with (p, t) = v[t*128+p]
    return np.ascontiguousarray(np.asarray(v, np.float32).reshape(ntiles, P).T)


def _prepare_maps(inputs):
    f32 = np.float32
    f16 = np.float16
    f8 = ml_dtypes.float8_e4m3fn
    g = {k: np.asarray(v, f32) for k, v in inputs.items()}

    def fold(p):
        w = g[f"{p}_wo"] @ g[f"{p}_wv"]
        b = g[f"{p}_wo"] @ g[f"{p}_bv"] + g[f"{p}_bo"]
        return w, b

    def w1pack(w1):
        # [F1P*128, 2, H2]; [kk*128+p, i, h] = fp8(S * w1[h, (2kk+i)*128+p])
        w1T = np.ascontiguousarray(S * w1.T)  # [E, H2]
        r = w1T.reshape(F1P, 2, P, H2).transpose(0, 2, 1, 3)
        return np.ascontiguousarray(r.reshape(F1P * P, 2, H2)).astype(f8)

    def w2pack(w2, f2p):
        # fp8 part [MT*128, 2*f2p, 128]:
        #   [m*128+p, 2kk+i, mc] = fp8(S * w2[m*128+mc, (2kk+i)*128+p])
        # f16 part [MT*128, nf16, 128]:
        #   [m*128+p, j, mc] = f16(S * w2[m*128+mc, (2*f2p+j)*128+p])
        nf16 = HT - 2 * f2p
        w2s = S * w2  # [E, H2]
        r = w2s.reshape(MT, P, HT, P)         # [m, mc, ht, p]
        r = r.transpose(0, 3, 2, 1)           # [m, p, ht, mc]
        q = np.ascontiguousarray(
            r[:, :, :2 * f2p, :].reshape(MT * P, 2 * f2p, P)).astype(f8)
        h = np.ascontiguousarray(
            r[:, :, 2 * f2p:, :].reshape(MT * P, nf16, P)).astype(f16)
        return q, h

    W1f, c1 = fold("v2n")
    W2f, c2 = fold("n2v")
    c_fix = W1f @ c2
    w2qv_a, w2hv_a = w2pack(g["fv_w2"], F2PV)
    w2qn_a, w2hn_a = w2pack(g["fn_w2"], F2PN)
    def kmaj(w):
        # [E, cols] -> [P, KT, cols]
        c = w.shape[1]
        return np.ascontiguousarray(
            w.reshape(KT, P, c).transpose(1, 0, 2))

    common = {
        "wvo1": kmaj(np.ascontiguousarray(W1f.T)).astype(f16),
        "wvo2": kmaj(np.ascontiguousarray(W2f.T)).astype(f16),
        "lnv": np.concatenate([_pvec(S * g["ln_v_g"], MT),
                               _pvec(S * g["ln_v_b"], MT)], axis=1),
        "lnn": np.concatenate([_pvec(S * g["ln_n_g"], MT),
                               _pvec(S * g["ln_n_b"], MT)], axis=1),
        "w1qv": w1pack(g["fv_w1"]), "b1v": _pvec(g["fv_b1"], HT),
        "w1qn": w1pack(g["fn_w1"]), "b1n": _pvec(g["fn_b1"], HT),
        "w2qv": w2qv_a, "w2hv": w2hv_a, "b2v": _pvec(S * g["fv_b2"], MT),
        "w2qn": w2qn_a, "w2hn": w2hn_a, "b2n": _pvec(S * g["fn_b2"], MT),
    }
    vT = kmaj(S * (g["verb_features"].T + (c1 - c_fix).reshape(E, 1))).astype(f16)
    nT = kmaj(S * (g["noun_features"].T + c2.reshape(E, 1))).astype(f16)
    in_maps = []
    for i in range(NCORES):
        cs = slice(i * B, (i + 1) * B)
        m = dict(common)
        m["vT"] = np.ascontiguousarray(vT[:, :, cs])
        m["nT"] = np.ascontiguousarray(nT[:, :, cs])
        in_maps.append(m)
    return in_maps


def kernel(**inputs):
    nc = _get_program()
    in_maps = _prepare_maps(inputs)
    res = run_bass_kernel_spmd(nc, in_maps, list(range(NCORES))).results
    verb = np.concatenate(
        [res[i]["verb_out"].astype(np.float32) for i in range(NCORES)], axis=1)
    noun = np.concatenate(
        [res[i]["noun_out"].astype(np.float32) for i in range(NCORES)], axis=1)
    return (np.ascontiguousarray(verb.T) * np.float32(1.0 / S),
            np.ascontiguousarray(noun.T) * np.float32(1.0 / S))


# revision 46
# speedup vs baseline: 1.0201x; 1.0151x over previous
"""Trainium2 Bass kernel for a dense cross-task transformer block (v2: fp8).

Math notes
----------
The reference "attention" has sequence length 1 on the key axis, so
softmax(scores) == 1.0 exactly and the whole q/k/score path is dead:

    mha_len1(q_in, kv_in, ...) == (kv_in @ wv.T + bv) @ wo.T + bo

which folds (on host) into a single matmul with W = wo @ wv and
b = wo @ bv + bo.  The block is then:

    verb1 = LN(verb + noun @ W1.T + c1)          (ln_v)
    verb2 = verb1 + FFN_v(verb1)
    noun1 = LN(noun + verb2 @ W2.T + c2)         (ln_n)
    noun2 = noun1 + FFN_n(noun1)
    return verb2, noun2

Bias folding: nT is pre-loaded as 16*(noun + c2) so it serves both as the
phase-C residual (which needs +c2) and the phase-A rhs; the resulting
spurious W1@c2 term in phase A is cancelled by pre-subtracting W1@c2 from
the verb residual on the host.  (All biases are zero in practice.)

Precision scheme (the kernel is PE-issue-bound, so FLOP dtype = speed):
 - Everything stored/streamed in fp16 (not bf16: 8x less rounding noise),
   activations and residuals carried at 16x scale so fp8-weight matmuls,
   f16-weight matmuls and residual adds all share one PSUM scale.  LN is
   scale-invariant, outputs are divided by 16 on the host.
 - FFN1 runs fully in fp8 e4m3 DoubleRow (2 contraction rows/cycle):
   lhsT = fp8(16*w1) pairs, rhs = fp8(LN output, true scale).
 - FFN2 contracts 2048: first F2P*256 rows in fp8 DoubleRow (gelu evacuates
   those h-tiles straight to fp8 pair tiles), the rest in f16.
 - Attention GEMMs stay f16 (their quantization noise is amplified ~2x
   through the LN + downstream path; FFN noise is the cheapest to spend).
   The fp8/f16 mix is chosen so total rel-err ~1.7e-2 < 2e-2 budget.

Engine choreography:
 - attention epilogues (PSUM evac, x^2, LN stats matmuls) are emitted one
   m-tile late so the PE never waits on DVE/ACT; stats PSUM is
   double-buffered across chunks (sx at partition 0, sqs at partition 32
   of one bank) so chunk c+1 never WARs on chunk c's stats reads.
 - LN mean/rstd broadcast via rank-1 matmuls, copied to f16 SBUF; the
   per-tile LN tail is 3 DVE ops + 1 ACT fp8 cast.
 - FFN weights stream per m-block; FFN1/FFN2 process chunk-pairs so each
   DoubleRow LDWEIGHTS (213ns, no FWL) amortizes over 2 matmuls.
"""

import os
import numpy as np
import ml_dtypes
from contextlib import ExitStack

import concourse.bass as bass
import concourse.bacc as bacc_mod
import concourse.mybir as mybir
import concourse.tile as tile
from concourse.bass_utils import run_bass_kernel_spmd

E = 1024          # embed dim
H2 = 2048         # FFN hidden dim
B_TOTAL = 16384
NCORES = 8
B = B_TOTAL // NCORES   # 2048 rows per core
P = 128
EPS = 1e-5
CH = 512          # column chunk
NCH = B // CH     # 4
KT = E // P       # 8
MT = E // P       # 8
HT = H2 // P      # 16
S = 16.0          # global activation/residual scale
F1P = 4           # FFN1 fp8 pairs (of KT//2=4): fully fp8
F2PV = 6          # verb FFN2 fp8 pairs (of HT//2=8)
F2PN = 5          # noun FFN2 fp8 pairs
F2PMAX = max(F2PV, F2PN)

F32 = mybir.dt.float32
F16 = mybir.dt.float16
F8 = mybir.dt.float8e4
AF = mybir.ActivationFunctionType
OP = mybir.AluOpType
DR = mybir.MatmulPerfMode.DoubleRow


def _build_program():
    nc = bacc_mod.Bacc("TRN2", target_bir_lowering=False)

    # activations/attention weights laid out [P, KT, cols] so bulk loads can
    # be single multi-k-tile DMA descriptors (startup is descriptor-bound)
    vT = nc.declare_dram_parameter("vT", [P, KT, B], F16, isOutput=False)
    nT = nc.declare_dram_parameter("nT", [P, KT, B], F16, isOutput=False)
    wvo1 = nc.declare_dram_parameter("wvo1", [P, KT, E], F16, isOutput=False)
    wvo2 = nc.declare_dram_parameter("wvo2", [P, KT, E], F16, isOutput=False)
    lnv = nc.declare_dram_parameter("lnv", [P, 2 * MT], F32, isOutput=False)  # 16g|16b
    lnn = nc.declare_dram_parameter("lnn", [P, 2 * MT], F32, isOutput=False)
    w1qv = nc.declare_dram_parameter("w1qv", [F1P * P, 2, H2], F8, isOutput=False)
    w1qn = nc.declare_dram_parameter("w1qn", [F1P * P, 2, H2], F8, isOutput=False)
    b1v = nc.declare_dram_parameter("b1v", [P, HT], F32, isOutput=False)
    b1n = nc.declare_dram_parameter("b1n", [P, HT], F32, isOutput=False)
    w2qv = nc.declare_dram_parameter("w2qv", [MT * P, 2 * F2PV, P], F8, isOutput=False)
    w2qn = nc.declare_dram_parameter("w2qn", [MT * P, 2 * F2PN, P], F8, isOutput=False)
    w2hv = nc.declare_dram_parameter("w2hv", [MT * P, HT - 2 * F2PV, P], F16,
                                     isOutput=False)
    w2hn = nc.declare_dram_parameter("w2hn", [MT * P, HT - 2 * F2PN, P], F16,
                                     isOutput=False)
    b2v = nc.declare_dram_parameter("b2v", [P, MT], F32, isOutput=False)  # 16*b2
    b2n = nc.declare_dram_parameter("b2n", [P, MT], F32, isOutput=False)
    verb_out = nc.declare_dram_parameter("verb_out", [E, B], F16, isOutput=True)
    noun_out = nc.declare_dram_parameter("noun_out", [E, B], F16, isOutput=True)

    with tile.TileContext(nc) as tc, ExitStack() as ctx:
        const = ctx.enter_context(tc.tile_pool(name="const", bufs=1))
        rhsp = ctx.enter_context(tc.tile_pool(name="rhsp", bufs=1))    # nT; later noun2'
        resp = ctx.enter_context(tc.tile_pool(name="resp", bufs=1))    # vT; later verb2'
        lnp = ctx.enter_context(tc.tile_pool(name="lnp", bufs=1))      # verb1'/noun1' f16
        l8p = ctx.enter_context(tc.tile_pool(name="l8p", bufs=1))      # verb1/noun1 fp8 pairs
        watp = ctx.enter_context(tc.tile_pool(name="watp", bufs=1))    # wvo1/wvo2
        w1p = ctx.enter_context(tc.tile_pool(name="w1p", bufs=1))      # w1 fp8 pair tiles
        w2qp = ctx.enter_context(tc.tile_pool(name="w2qp", bufs=3))    # w2 fp8 m-block stream
        w2hp = ctx.enter_context(tc.tile_pool(name="w2hp", bufs=3))    # w2 f16 m-block stream
        hp = ctx.enter_context(tc.tile_pool(name="hp", bufs=1))        # gelu hidden
        sqp = ctx.enter_context(tc.tile_pool(name="sqp", bufs=1))      # x^2 (8 tags)
        smp = ctx.enter_context(tc.tile_pool(name="smp", bufs=1))      # LN stats vectors
        bsp = ctx.enter_context(tc.tile_pool(name="bsp", bufs=3))      # bcast SBUF copies
        mmp = ctx.enter_context(tc.tile_pool(name="mmp", bufs=4, space="PSUM"))
        stp = ctx.enter_context(tc.tile_pool(name="stp", bufs=2, space="PSUM"))
        bcp = ctx.enter_context(tc.tile_pool(name="bcp", bufs=1, space="PSUM"))

        ones_col = const.tile([P, 1], F16, tag="ones_col", name="ones_col")
        nc.vector.memset(ones_col[:], 1.0)
        ones_row = const.tile([1, P], F16, tag="ones_row", name="ones_row")
        nc.vector.memset(ones_row[:], 1.0)
        eps_t = const.tile([1, 1], F32, tag="eps", name="eps")
        nc.vector.memset(eps_t[:], EPS * S * S)

        def load_const(dram, shape, tag, dtype=F32):
            t = const.tile(shape, dtype, tag=tag, name=tag)
            nc.scalar.dma_start(out=t[:], in_=dram[:, :])
            return t

        def chunk_tiles(pool, pfx):
            return [[pool.tile([P, CH], F16, tag=f"{pfx}{k}c{c}", name=f"{pfx}{k}c{c}")
                     for c in range(NCH)] for k in range(KT)]

        def pair_tiles(pool, pfx):
            return [pool.tile([P, 2, B], F8, tag=f"{pfx}{kk}", name=f"{pfx}{kk}")
                    for kk in range(KT // 2)]

        def full_tiles(pool, pfx):
            # one big tile so bulk loads can be single multi-k DMA descriptors
            big = pool.tile([P, KT, B], F16, tag=f"{pfx}big", name=f"{pfx}big")
            return [big[:, k, :] for k in range(KT)], big

        def load_wat(dram):
            big = watp.tile([P, KT, E], F16, tag="wabig", name="wabig")
            for k in range(KT):
                eng = nc.scalar if k % 2 == 0 else nc.sync
                eng.dma_start(out=big[:, k, :], in_=dram[:, k, :])
            return [big[:, k, :] for k in range(KT)]

        def load_w1q(dram):
            tiles = []
            for kk in range(F1P):
                t = w1p.tile([P, 2, H2], F8, tag=f"w1_{kk}", name=f"w1_{kk}")
                eng = nc.scalar if kk % 2 == 0 else nc.sync
                eng.dma_start(out=t[:, :, :], in_=dram[kk * P:(kk + 1) * P, :, :])
                tiles.append(t)
            return tiles

        def attn_phase(wt, rhs, resid, out16t, out8t, gb_pb, k_outer_c0=False):
            """resid[m][:,cs] <- xt' = resid + wt.T@rhs (16x, f16, in place);
            out16t[m][c] <- 16*LN(x) f16; out8t[kk][:,m%2,cs] <- fp8 LN(x).

            Pipelined epilogues: the PSUM-evac add trails its matmul group by
            one m-slot, the x^2 + stats matmuls by two (so the PE's in-order
            queue never waits on DVE/ACT), and the per-chunk LN post work is
            sliced per-m and drip-fed one item per slot.  Unfinished items are
            returned as a thunk list the next phase interleaves into itself.
            """
            g_pb = gb_pb[:, 0:MT]
            b_pb = gb_pb[:, MT:2 * MT]
            seq = [(c, m) for c in range(NCH) for m in range(MT)]
            stash = {}
            headq = []
            tailq = []

            def add_item(s):
                c, m, ps, st = stash[s]
                cs = slice(c * CH, (c + 1) * CH)
                xt = resid[m][:, cs]
                nc.vector.tensor_add(xt, ps[:], xt)

            def sqstat_item(s):
                # x^2 per m-slot (spread over the ACT/DVE queues); the 16
                # stats matmuls are emitted as one per-chunk batch so the PE
                # pays the array-reconfig hiccup twice per chunk, not 16x.
                c, m, ps, st = stash.pop(s)
                cs = slice(c * CH, (c + 1) * CH)
                xt = resid[m][:, cs]
                sq = sqp.tile([P, CH], F16, tag=f"sq{m}", name=f"sq{m}")
                sqt[m] = sq
                if flush_mode[0]:
                    nc.vector.tensor_mul(sq[:], xt, xt)
                else:
                    nc.scalar.activation(sq[:], xt, AF.Square)
                if m == MT - 1:
                    headq.insert(0, lambda: stats_batch(c, st))

            def stats_batch(c, st):
                cs = slice(c * CH, (c + 1) * CH)
                for m in range(MT):
                    nc.tensor.matmul(st[0:1, :], lhsT=ones_col[:],
                                     rhs=resid[m][:, cs],
                                     start=(m == 0), stop=(m == MT - 1))
                    nc.tensor.matmul(st[32:33, :], lhsT=ones_col[:],
                                     rhs=sqt[m][:],
                                     start=(m == 0), stop=(m == MT - 1))
                headq.append(lambda: post_head(c, st))
                for mm in range(MT):
                    tailq.append(lambda c=c, mm=mm: post_tail(c, mm))

            sqt = {}

            def post_bcast(c, nm, rsb):
                dve = flush_mode[0]
                nmB = bcp.tile([P, CH], F32, tag="nmB", name="nmB")
                nc.tensor.matmul(nmB[:], lhsT=ones_row[:], rhs=nm[:],
                                 start=True, stop=True)
                rsB = bcp.tile([P, CH], F32, tag="rsB", name="rsB")
                nc.tensor.matmul(rsB[:], lhsT=ones_row[:], rhs=rsb[:],
                                 start=True, stop=True)
                nmS = bsp.tile([P, CH], F16, tag="nmS", name="nmS")
                rsS = bsp.tile([P, CH], F16, tag="rsS", name="rsS")
                if dve:
                    nc.vector.tensor_scalar(nmS[:], nmB[:], 1.0, None, OP.mult)
                    nc.vector.tensor_scalar(rsS[:], rsB[:], 1.0, None, OP.mult)
                else:
                    nc.scalar.activation(nmS[:], nmB[:], AF.Copy)
                    nc.scalar.activation(rsS[:], rsB[:], AF.Copy)
                bsp_by_chunk[c] = (nmS, rsS)

            def post_head(c, st):
                # In flush mode (drained inside the next FFN phase) the whole
                # chain runs on DVE — table-free rsqrt via magic-constant
                # seed + 2 Newton steps — so the ACT queue never switches
                # functions mid-Gelu-stream (a switch costs a 1.28us table
                # reload and stalls the PSUM evacuation).  The broadcast
                # matmuls are deferred to the tail queue so the PE reaches
                # them after the DVE chain has finished.
                dve = flush_mode[0]
                nm = smp.tile([1, CH], F16, tag="nm", name="nm")
                t1 = smp.tile([1, CH], F32, tag="t1", name="t1")
                m2 = smp.tile([1, CH], F32, tag="m2", name="m2")
                rsb = smp.tile([1, CH], F16, tag="rsb", name="rsb")
                if dve:
                    nc.vector.tensor_scalar(nm[:], st[0:1, :], -1.0 / E, None,
                                            OP.mult)
                    nc.vector.tensor_scalar(t1[:], st[32:33, :], 1.0 / E, None,
                                            OP.mult)
                    nc.vector.tensor_mul(m2[:], nm[:], nm[:])
                    nc.vector.tensor_sub(t1[:], t1[:], m2[:])      # 256*var
                    nc.vector.tensor_scalar(t1[:], t1[:], EPS * S * S, None,
                                            OP.add)
                    # magic-constant rsqrt seed; the DVE ALU adds in fp32, so
                    # MAGIC - (bits>>1) is computed as a value-domain mul-add
                    # into a u32 tile (values < 2^31, +-64 bit-error is
                    # irrelevant for a 3%-accurate seed)
                    y0u = smp.tile([1, CH], mybir.dt.uint32, tag="y0u",
                                   name="y0u")
                    nc.vector.tensor_scalar(
                        y0u[:], t1[:].bitcast(mybir.dt.uint32), 1, None,
                        OP.logical_shift_right)
                    nc.vector.tensor_scalar(y0u[:], y0u[:], -1.0,
                                            float(0x5f3759df), OP.mult, OP.add)
                    y = y0u[:].bitcast(F32)
                    tt = smp.tile([1, CH], F32, tag="tt", name="tt")
                    for _ in range(2):
                        nc.vector.tensor_mul(tt[:], y, y)
                        nc.vector.tensor_mul(tt[:], tt[:], t1[:])
                        nc.vector.tensor_scalar(tt[:], tt[:], -0.5, 1.5,
                                                OP.mult, OP.add)
                        nc.vector.tensor_mul(y, y, tt[:])
                    nc.vector.tensor_scalar(rsb[:], y, 1.0, None, OP.mult)
                    headq.append(lambda: post_bcast(c, nm, rsb))
                else:
                    nc.scalar.activation(nm[:], st[0:1, :], AF.Copy,
                                         scale=-1.0 / E)
                    nc.scalar.activation(t1[:], st[32:33, :], AF.Copy,
                                         scale=1.0 / E)
                    nc.scalar.activation(m2[:], nm[:], AF.Square)
                    nc.vector.tensor_sub(t1[:], t1[:], m2[:])      # 256*var
                    nc.scalar.activation(t1[:], t1[:], AF.Sqrt, bias=eps_t[:])
                    rs = smp.tile([1, CH], F32, tag="rs", name="rs")
                    nc.vector.reciprocal_approx_fast(out=rs[:], in_=t1[:])
                    nc.scalar.activation(rsb[:], rs[:], AF.Copy)
                    headq.append(lambda: post_bcast(c, nm, rsb))

            def post_tail(c, m):
                cs = slice(c * CH, (c + 1) * CH)
                nmS, rsS = bsp_by_chunk[c]
                xt = resid[m][:, cs]
                tmp = bsp.tile([P, CH], F16, tag="tmp", name="tmp")
                nc.vector.tensor_add(tmp[:], xt, nmS[:])
                nc.vector.tensor_mul(tmp[:], tmp[:], rsS[:])
                o16 = out16t[m][c]
                nc.vector.tensor_scalar(
                    o16[:], tmp[:], g_pb[:, m:m + 1], b_pb[:, m:m + 1],
                    OP.mult, OP.add)
                o8 = out8t[m // 2][:, m % 2, cs]
                if flush_mode[0]:
                    nc.vector.tensor_scalar(o8, o16[:], 1.0 / S, None, OP.mult)
                else:
                    nc.scalar.activation(o8, o16[:], AF.Copy, scale=1.0 / S)

            bsp_by_chunk = {}
            flush_mode = [False]

            def deferred(s):
                if s - 1 >= 0:
                    add_item(s - 1)
                if s - 2 >= 0:
                    sqstat_item(s - 2)
                if headq:
                    headq.pop(0)()
                elif tailq:
                    tailq.pop(0)()
                # 11 deferred items per 8 slots: drain an extra LN tail when
                # the queue backs up so no chunk<=1 fp8 cast leaks into the
                # flush (FFN1 reads chunks 0/1 immediately)
                if len(tailq) > 8:
                    tailq.pop(0)()

            sts = {}
            for s, (c, m) in enumerate(seq):
                if m == 0:
                    sts[c] = stp.tile([64, CH], F32, tag="st", name="st")
                st = sts[c]
                cs = slice(c * CH, (c + 1) * CH)
                if c == 0 and k_outer_c0 and m % 4 == 0:
                    # k-outer over m-quads: start computing as soon as the
                    # first (weight, rhs) k-tile pair lands from DRAM
                    half = range(m, m + 4)
                    pss = {mm: mmp.tile([P, CH], F32, tag="mm", name="mm")
                           for mm in half}
                    for k in range(KT):
                        for mm in half:
                            nc.tensor.matmul(
                                pss[mm][:], lhsT=wt[k][:, mm * P:(mm + 1) * P],
                                rhs=rhs[k][:, cs],
                                start=(k == 0), stop=(k == KT - 1))
                    for mm in half:
                        stash[s + mm - m] = (c, mm, pss[mm], st)
                elif c == 0 and k_outer_c0:
                    pass          # emitted with the quad above
                else:
                    ps = mmp.tile([P, CH], F32, tag="mm", name="mm")
                    for k in range(KT):
                        nc.tensor.matmul(
                            ps[:], lhsT=wt[k][:, m * P:(m + 1) * P],
                            rhs=rhs[k][:, cs],
                            start=(k == 0), stop=(k == KT - 1))
                    stash[s] = (c, m, ps, st)
                deferred(s)

            n = len(seq)

            def set_flush():
                flush_mode[0] = True

            def drainer():
                if headq:
                    headq.pop(0)()
                elif tailq:
                    tailq.pop(0)()

            # head part: the last adds/stats plus the final chunk's DVE-only
            # LN head (table-free rsqrt); tail part: the deferred broadcast
            # matmuls + LN tails, dripped into the next phase's slots.
            backlog = len(tailq)
            head_items = [set_flush,
                          lambda: add_item(n - 1),
                          lambda: sqstat_item(n - 2),
                          lambda: sqstat_item(n - 1),
                          drainer]                      # runs post_head
            tail_items = [drainer] * (3 + backlog + MT)
            return head_items, tail_items

        def ffn_phase(in16, in8, w1q_t, w2q_dram, w2h_dram, b1_pb, b2_pb,
                      residt, out_dram, f2p, pre_flush=()):
            nf16 = HT - 2 * f2p
            """residt[m][:,cs] <- 16*(LN_out + FFN(LN_out)) f16, DMA'd out.

            FFN1 fully fp8 DoubleRow; FFN2 contracts F2P*256 rows fp8 + rest
            f16.  Chunk-pair blocking so every LDWEIGHTS covers 2 matmuls.
            The previous phase's leftover thunks are drip-fed one per slot;
            FFN2 weight m-blocks are DMA-prefetched two m ahead.
            """
            head_items, tail_items = (list(pre_flush[0]), list(pre_flush[1])) \
                if pre_flush else ([], [])

            def pump():
                if tail_items:
                    tail_items.pop(0)()

            def dma_w2(m):
                wq = w2qp.tile([P, 2 * F2PMAX, P], F8, tag="w2q", name="w2q")
                (nc.scalar if m % 2 == 0 else nc.sync).dma_start(
                    out=wq[:, 0:2 * f2p, :], in_=w2q_dram[m * P:(m + 1) * P, :, :])
                wh = w2hp.tile([P, HT - 2 * min(F2PV, F2PN), P], F16,
                               tag="w2h", name="w2h")
                (nc.sync if m % 2 == 0 else nc.scalar).dma_start(
                    out=wh[:, 0:nf16, :], in_=w2h_dram[m * P:(m + 1) * P, :, :])
                return wq, wh

            for cb in range(NCH // 2):
                css = [slice((2 * cb + ci) * CH, (2 * cb + ci + 1) * CH)
                       for ci in range(2)]
                h8 = [[hp.tile([P, 2, CH], F8, tag=f"h8_{kk}_{ci}",
                               name=f"h8_{kk}_{ci}") for ci in range(2)]
                      for kk in range(f2p)]
                h16 = [[hp.tile([P, CH], F16, tag=f"h16_{j}_{ci}",
                                name=f"h16_{j}_{ci}") for ci in range(2)]
                       for j in range(nf16)]
                w2t = {}
                for hm in range(HT):
                    pss = [mmp.tile([P, CH], F32, tag="mm", name="mm")
                           for _ in range(2)]
                    for kk in range(F1P):
                        for ci in range(2):
                            nc.tensor.matmul(
                                pss[ci][:], lhsT=w1q_t[kk][:, :, hm * P:(hm + 1) * P],
                                rhs=in8[kk][:, :, css[ci]],
                                start=(kk == 0), stop=(kk == F1P - 1),
                                perf_mode=DR)
                    for ci in range(2):
                        if hm < 2 * f2p:
                            dst = h8[hm // 2][ci][:, hm % 2, :]
                        else:
                            dst = h16[hm - 2 * f2p][ci][:]
                        nc.scalar.activation(dst, pss[ci][:], AF.Gelu,
                                             bias=b1_pb[:, hm:hm + 1],
                                             scale=1.0 / S)
                    if cb == 0 and hm == 0:
                        # previous phase's deferred adds/stats + its DVE-only
                        # LN head (no ACT-table traffic) drain here
                        for it in head_items:
                            it()
                        head_items = []
                    elif hm >= 5:
                        pump()
                    if hm == HT - 4:
                        w2t[0] = dma_w2(0)
                    elif hm == HT - 2:
                        w2t[1] = dma_w2(1)
                for m in range(MT):
                    if m + 2 < MT:
                        w2t[m + 2] = dma_w2(m + 2)
                    wq, wh = w2t.pop(m)
                    pss = [mmp.tile([P, CH], F32, tag="mm", name="mm")
                           for _ in range(2)]
                    for kk in range(f2p):
                        for ci in range(2):
                            nc.tensor.matmul(
                                pss[ci][:], lhsT=wq[:, 2 * kk:2 * kk + 2, :],
                                rhs=h8[kk][ci][:, :, :],
                                start=(kk == 0),
                                stop=(kk == f2p - 1 and nf16 == 0),
                                perf_mode=DR)
                    for j in range(nf16):
                        for ci in range(2):
                            nc.tensor.matmul(
                                pss[ci][:], lhsT=wh[:, j, :],
                                rhs=h16[j][ci][:],
                                start=(f2p == 0 and j == 0),
                                stop=(j == nf16 - 1))
                    for ci in range(2):
                        ot = residt[m][:, css[ci]]
                        nc.vector.affine_then_add(
                            ot, pss[ci][:], in16[m][2 * cb + ci][:],
                            scale=1.0, bias=b2_pb[:, m:m + 1])
                        (nc.sync if ci == 0 else nc.scalar).dma_start(
                            out=out_dram[m * P:(m + 1) * P, css[ci]], in_=ot)
                    pump()
            for it in head_items:
                it()
            while tail_items:
                tail_items.pop(0)()

        _REP = int(os.environ.get("BENCH_REPEAT", "1"))
        for _rep in range(_REP):
            # ---- phase A: verb attends to noun, LN -> verb1 ----
            # DMA order follows first-chunk consumption: the c0 m-quad only
            # needs wa[k][:,0:512] + nt[k][:,0:512] per k, so those stream
            # first (sync + vector queues: the scalar queue is blocked for
            # ~11.5us by the startup ACT table loads), then the second wa
            # half (quad 2), then vt chunk 0 (chunk-0 epilogues), then the
            # bulk.
            nt_t, ntb = full_tiles(rhsp, "n")    # 16*(noun+c2); phase-C residual
            vt_t, vtb = full_tiles(resp, "v")    # 16*(verb + c1 - W1@c2)
            wab = watp.tile([P, KT, E], F16, tag="wabig", name="wabig")
            wa1 = [wab[:, k, :] for k in range(KT)]
            # critical stream feeding the first chunk's k-outer quads:
            # per-k descriptors on two queues, everything else batched
            for k in range(KT):
                nc.sync.dma_start(out=wab[:, k, 0:CH], in_=wvo1[:, k, 0:CH])
                nc.gpsimd.dma_start(out=ntb[:, k, 0:CH], in_=nT[:, k, 0:CH])
            for k in range(KT):
                nc.sync.dma_start(out=wab[:, k, CH:E], in_=wvo1[:, k, CH:E])
                nc.gpsimd.dma_start(out=vtb[:, k, 0:CH], in_=vT[:, k, 0:CH])
            lnv_pb = load_const(lnv, [P, 2 * MT], "lnv")
            lnn_pb = load_const(lnn, [P, 2 * MT], "lnn")
            b1v_pb = load_const(b1v, [P, HT], "b1v")
            b2v_pb = load_const(b2v, [P, MT], "b2v")
            b1n_pb = load_const(b1n, [P, HT], "b1n")
            b2n_pb = load_const(b2n, [P, MT], "b2n")
            for k in range(KT):
                eng = nc.sync if k % 2 == 0 else nc.scalar
                eng.dma_start(out=ntb[:, k, CH:B], in_=nT[:, k, CH:B])
            for k in range(KT):
                eng = nc.sync if k % 2 == 0 else nc.scalar
                eng.dma_start(out=vtb[:, k, CH:B], in_=vT[:, k, CH:B])
            w1v_t = load_w1q(w1qv)               # prefetch for phase B
            verb1 = chunk_tiles(lnp, "l")
            v1q = pair_tiles(l8p, "q")
            fl_a = attn_phase(wa1, nt_t, vt_t, verb1, v1q, lnv_pb,
                              k_outer_c0=True)

            # ---- phase B: verb FFN -> verb2' (written into the vT tiles) ----
            wa2 = load_wat(wvo2)                 # prefetch for phase C
            ffn_phase(verb1, v1q, w1v_t, w2qv, w2hv, b1v_pb, b2v_pb,
                      vt_t, verb_out, F2PV, pre_flush=fl_a)

            # ---- phase C: noun attends to verb2, LN -> noun1 ----
            w1n_t = load_w1q(w1qn)               # prefetch for phase D
            noun1 = chunk_tiles(lnp, "l")
            n1q = pair_tiles(l8p, "q")
            fl_c = attn_phase(wa2, vt_t, nt_t, noun1, n1q, lnn_pb)

            # ---- phase D: noun FFN -> noun2' (written into the nT tiles) ----
            ffn_phase(noun1, n1q, w1n_t, w2qn, w2hn, b1n_pb, b2n_pb,
                      nt_t, noun_out, F2PN, pre_flush=fl_c)

    nc.finalize()
    return nc


_prog_cache = {}


def _get_program():
    if "nc" not in _prog_cache:
        _prog_cache["nc"] = _build_program()
    return _prog_cache["nc"]


def _pvec(v, ntiles):
    # [ntiles*128] -> [128, ntiles] with (p, t) = v[t*128+p]
    return np.ascontiguousarray(np.asarray(v, np.float32).reshape(ntiles, P).T)


def _prepare_maps(inputs):
    f32 = np.float32
    f16 = np.float16
    f8 = ml_dtypes.float8_e4m3fn
    g = {k: np.asarray(v, f32) for k, v in inputs.items()}

    def fold(p):
        w = g[f"{p}_wo"] @ g[f"{p}_wv"]
        b = g[f"{p}_wo"] @ g[f"{p}_bv"] + g[f"{p}_bo"]
        return w, b

    def w1pack(w1):
        # [F1P*128, 2, H2]; [kk*128+p, i, h] = fp8(S * w1[h, (2kk+i)*128+p])
        w1T = np.ascontiguousarray(S * w1.T)  # [E, H2]
        r = w1T.reshape(F1P, 2, P, H2).transpose(0, 2, 1, 3)
        return np.ascontiguousarray(r.reshape(F1P * P, 2, H2)).astype(f8)

    def w2pack(w2, f2p):
        # fp8 part [MT*128, 2*f2p, 128]:
        #   [m*128+p, 2kk+i, mc] = fp8(S * w2[m*128+mc, (2kk+i)*128+p])
        # f16 part [MT*128, nf16, 128]:
        #   [m*128+p, j, mc] = f16(S * w2[m*128+mc, (2*f2p+j)*128+p])
        nf16 = HT - 2 * f2p
        w2s = S * w2  # [E, H2]
        r = w2s.reshape(MT, P, HT, P)         # [m, mc, ht, p]
        r = r.transpose(0, 3, 2, 1)           # [m, p, ht, mc]
        q = np.ascontiguousarray(
            r[:, :, :2 * f2p, :].reshape(MT * P, 2 * f2p, P)).astype(f8)
        h = np.ascontiguousarray(
            r[:, :, 2 * f2p:, :].reshape(MT * P, nf16, P)).astype(f16)
        return q, h

    W1f, c1 = fold("v2n")
    W2f, c2 = fold("n2v")
    c_fix = W1f @ c2
    w2qv_a, w2hv_a = w2pack(g["fv_w2"], F2PV)
    w2qn_a, w2hn_a = w2pack(g["fn_w2"], F2PN)
    def kmaj(w):
        # [E, cols] -> [P, KT, cols]
        c = w.shape[1]
        return np.ascontiguousarray(
            w.reshape(KT, P, c).transpose(1, 0, 2))

    common = {
        "wvo1": kmaj(np.ascontiguousarray(W1f.T)).astype(f16),
        "wvo2": kmaj(np.ascontiguousarray(W2f.T)).astype(f16),
        "lnv": np.concatenate([_pvec(S * g["ln_v_g"], MT),
                               _pvec(S * g["ln_v_b"], MT)], axis=1),
        "lnn": np.concatenate([_pvec(S * g["ln_n_g"], MT),
                               _pvec(S * g["ln_n_b"], MT)], axis=1),
        "w1qv": w1pack(g["fv_w1"]), "b1v": _pvec(g["fv_b1"], HT),
        "w1qn": w1pack(g["fn_w1"]), "b1n": _pvec(g["fn_b1"], HT),
        "w2qv": w2qv_a, "w2hv": w2hv_a, "b2v": _pvec(S * g["fv_b2"], MT),
        "w2qn": w2qn_a, "w2hn": w2hn_a, "b2n": _pvec(S * g["fn_b2"], MT),
    }
    vT = kmaj(S * (g["verb_features"].T + (c1 - c_fix).reshape(E, 1))).astype(f16)
    nT = kmaj(S * (g["noun_features"].T + c2.reshape(E, 1))).astype(f16)
    in_maps = []
    for i in range(NCORES):
        cs = slice(i * B, (i + 1) * B)
        m = dict(common)
        m["vT"] = np.ascontiguousarray(vT[:, :, cs])
        m["nT"] = np.ascontiguousarray(nT[:, :, cs])
        in_maps.append(m)
    return in_maps


def kernel(**inputs):
    nc = _get_program()
    in_maps = _prepare_maps(inputs)
    res = run_bass_kernel_spmd(nc, in_maps, list(range(NCORES))).results
    verb = np.concatenate(
        [res[i]["verb_out"].astype(np.float32) for i in range(NCORES)], axis=1)
    noun = np.concatenate(
        [res[i]["noun_out"].astype(np.float32) for i in range(NCORES)], axis=1)
    return (np.ascontiguousarray(verb.T) * np.float32(1.0 / S),
            np.ascontiguousarray(noun.T) * np.float32(1.0 / S))
